# revision 2
# baseline (speedup 1.0000x reference)
"""Trainium2 Bass kernel v2 for the MACE 3-body block (flipped orientation).

Design notes (vs baseline, which measured ~134us/iter):
  - All engine elementwise ops need partition base in {0,32,64,96} and only
    DVE/Act can read PSUM (gpsimd cannot), so the layout keeps every
    PSUM-consuming op on DVE and every SBUF-only mul on gpsimd.
  - f = n*64 + c (n-major).  fc1 and fc2 are FLIPPED matmuls (stationary
    lhsT = data slab, moving rhs = weights), so their outputs land with f
    on partitions and no [c,(m,n)] -> [m,(c,n)] gpsimd flatten is needed;
    biases ride along as an extra ones-row in lhsT (fc1) or per-partition
    Act bias (fc2, partitions are (u,d)).
  - Monomials: m2F81 row 9j+i = x_i*x_j (padded square; U2 coeffs only on
    i<=j rows).  Triples t=(j,i,k) i<=j<=k, j-blocks, (i,k) i-major.
    mon_lo = m3[0:128]; mon_hi = [m2F81(81) | x9(9) | xdup(1) | m3[128:](37)]
    -- exactly 128 rows each, no garbage partitions.
  - Replications are DMA-only (whole-F gpsimd muls stay SBUF-only):
    xpre/xsuf/xrep broadcast-APs from a DRAM x stage (HW-verified pattern),
    m2rep from SBUF with partition-stride-1 + inner 0-stride repeat APs.
  - Main contraction per 128-wide f-tile: D^T[f,198] = mon_slab^T @ U_dev
    (2 accumulating matmuls, lhsT = mon slab).  D^T slots pack 8-per-4-bank
    PSUM region (256-f32 stride -> no slot crosses a bank).
  - w-apply + path-sum fused on DVE: one tensor_mul (D^T * wgT with a
    0-stride o-repeat read of the per-node path weights) + one
    tensor_reduce(axis=X) over the 22 paths per o.  No wrep broadcast, no
    G matmul, no PSUM->SBUF term copies, no output-side DRAM flip.
  - fc2: per-l block-diagonal [128,128] lhsT over partitions (u,c), rhs
    selects (j,o) columns of termF16 via strided APs; output [128,576]
    f32 = (u,d) x (o-major cols), host unpacks.
"""

import numpy as np

# ---------------------------------------------------------------- constants
NTOT, MD, CD = 1024, 9, 64
P3D, P2D, P1D = 16, 4, 2
PPC = P3D + P2D + P1D               # 22 paths per output m
NCORES = 8
NLOC = NTOT // NCORES
MOUT = MD * PPC                     # 198

PAIRS_JI = [(j, i) for j in range(MD) for i in range(j + 1)]      # 45
TRIPS_V2 = [(j, i, k) for j in range(MD) for i in range(j + 1)
            for k in range(j, MD)]                                # 165
N3 = len(TRIPS_V2)
BJ = [0]
for j in range(MD):
    BJ.append(BJ[-1] + (j + 1) * (MD - j))
M3SPLIT = 128
HI_M2, HI_X, HI_DUP, HI_M3B = 0, 81, 90, 91

NSLOT = 8                           # D^T slots per PSUM group
SLOTW = 256                         # f32 stride between slots (1KB)

_PK_ITEMS = (("uclo", MOUT), ("uchi", MOUT), ("w1t", 3 * CD),
             ("wc", 3 * 128))
PK_OFF = {}
_c = 0
for _nm, _w in _PK_ITEMS:
    PK_OFF[_nm] = _c
    _c += _w
PK_BASE = _c                        # xt starts here (65 rows: 64 c + ones)

_PROGRAM = {}


# ---------------------------------------------------------------- host prep
def _sym_compress(U3, U2):
    tidx = {}
    for t, (j, i, k) in enumerate(TRIPS_V2):
        tidx[(i, j, k)] = t
    qidx = {}
    for q, (j, i) in enumerate(PAIRS_JI):
        qidx[(i, j)] = q
    U3c = np.zeros((MD, N3, P3D), np.float64)
    for a in range(MD):
        for b in range(MD):
            for i in range(MD):
                U3c[:, tidx[tuple(sorted((a, b, i)))], :] += U3[:, a, b, i, :]
    U2c = np.zeros((MD, len(PAIRS_JI), P2D), np.float64)
    for a in range(MD):
        for i in range(MD):
            U2c[:, qidx[tuple(sorted((a, i)))], :] += U2[:, a, i, :]
    return U3c.astype(np.float32), U2c.astype(np.float32)


def _build_u_dev(U3c, U2c, U1):
    """U_dev [256, 198]; cols = o*22 + pp, pp = [p3(16) | p2(4) | p1(2)].
    Rows = [mon_lo 0..127 | mon_hi 0..127]."""
    U = np.zeros((256, MOUT), np.float32)

    def col(o, pp):
        return o * PPC + pp

    for t in range(M3SPLIT):
        for o in range(MD):
            U[t, col(o, 0):col(o, P3D)] = U3c[o, t, :]
    for q, (j, i) in enumerate(PAIRS_JI):
        for o in range(MD):
            U[128 + HI_M2 + 9 * j + i,
              col(o, P3D):col(o, P3D + P2D)] = U2c[o, q, :]
    for m in range(MD):
        for o in range(MD):
            U[128 + HI_X + m, col(o, P3D + P2D):col(o, PPC)] = U1[o, m, :]
    for t in range(M3SPLIT, N3):
        for o in range(MD):
            U[128 + HI_M3B + (t - M3SPLIT), col(o, 0):col(o, P3D)] = U3c[o, t, :]
    return U


# ---------------------------------------------------------------- device
def _build_program(nloc, repeat=1):
    import concourse.bacc as bacc
    import concourse.bass as bass
    from concourse import mybir
    from concourse.tile import TileContext

    f16 = mybir.dt.float16
    f32 = mybir.dt.float32
    AF = mybir.ActivationFunctionType
    AX = mybir.AxisListType
    ALU = mybir.AluOpType
    F = nloc * CD                   # 8192
    NJT = F // 128                  # 64 f-tiles
    NGRP = NJT // NSLOT             # 8 PSUM groups
    w9 = MD * nloc
    wob = [1, 3, 5]                 # o-widths per l

    nc = bacc.Bacc("TRN2", debug=False, enable_asserts=False,
                   num_devices=NCORES, num_swdge_queues=4)

    pkw = PK_BASE + w9
    pk_d = nc.dram_tensor("pk", [128, pkw], f16, kind="ExternalInput").ap()
    b2_d = nc.dram_tensor("b2", [128, 1], f32, kind="ExternalInput").ap()
    wgt_d = nc.dram_tensor("wgt", [128, NJT * PPC], f16,
                           kind="ExternalInput").ap()
    out_d = nc.dram_tensor("out", [128, MD * CD], f32,
                           kind="ExternalOutput").ap()

    def dap(t, off, pattern):
        return bass.AP(tensor=t.tensor, offset=t.offset + off, ap=pattern)

    with TileContext(nc) as tc:
        with (
            tc.tile_pool(name="const", bufs=1) as const,
            tc.tile_pool(name="big", bufs=1) as big,
            tc.tile_pool(name="dram", bufs=1, space="DRAM") as dpool,
            tc.tile_pool(name="work", bufs=2) as work,
        ):
          _dmaq = [0]

          def dma(out, in_):
              i = _dmaq[0] % 2
              _dmaq[0] += 1
              (nc.sync if i == 0 else nc.scalar).dma_start(out=out, in_=in_)

          def _emit():
            pk = const.tile([128, pkw], f16, name="pk")
            sb_uclo = pk[0:128, PK_OFF["uclo"]:PK_OFF["uclo"] + MOUT]
            sb_uchi = pk[0:128, PK_OFF["uchi"]:PK_OFF["uchi"] + MOUT]
            sb_w1t = pk[0:65, PK_OFF["w1t"]:PK_OFF["w1t"] + 3 * CD]
            sb_wc = pk[0:128, PK_OFF["wc"]:PK_OFF["wc"] + 3 * 128]
            sb_xt = pk[0:65, PK_BASE:PK_BASE + w9]
            b2sb = const.tile([128, 1], f32, name="b2sb")
            wgt = const.tile([128, NJT * PPC], f16, name="wgt")
            nc.sync.dma_start(out=pk[:], in_=pk_d)
            nc.scalar.dma_start(out=b2sb[:], in_=b2_d)
            nc.sync.dma_start(out=wgt[:], in_=wgt_d)

            # ---------------- fc1 flipped: ysbT [128 n, (m, c)]
            ysbT = big.tile([128, MD * CD], f16, name="ysbT")
            with tc.tile_pool(name="ps_fc1", bufs=1, space="PSUM") as ps_fc1:
                psA = ps_fc1.tile([128, 512], f32, name="psA")
                psB = ps_fc1.tile([128, CD], f32, name="psB")
                for m in range(MD):
                    l = 0 if m == 0 else (1 if m < 4 else 2)
                    k1 = 65 if m == 0 else CD
                    dst = psA[:, m * CD:(m + 1) * CD] if m < 8 else psB[:]
                    nc.tensor.matmul(
                        dst, lhsT=sb_xt[0:k1, m * nloc:(m + 1) * nloc],
                        rhs=sb_w1t[0:k1, l * CD:(l + 1) * CD],
                        start=True, stop=True)
                nc.scalar.activation(ysbT[:, 0:512], psA[:], AF.Copy)
                nc.scalar.activation(ysbT[:, 512:576], psB[:], AF.Copy)

            # ---------------- x stage via DRAM
            ysbT_dram = dpool.tile([128, MD * CD], f16, name="ysbT_dram")
            nc.sync.dma_start(out=ysbT_dram[:], in_=ysbT[:])
            mon_hi = big.tile([128, F], f16, name="mon_hi")
            mon_lo = big.tile([128, F], f16, name="mon_lo")
            # x rows: for m: for n: for c: addr = n*576 + m*64 + c
            x_dram = dpool.tile([MD, F], f16, name="x_dram")
            nc.sync.dma_start(
                out=x_dram[:],
                in_=dap(ysbT_dram, 0, [[CD, MD], [MD * CD, nloc], [1, CD]]))
            nc.scalar.dma_start(
                out=mon_hi[HI_X:HI_X + MD, :],
                in_=dap(ysbT_dram, 0, [[CD, MD], [MD * CD, nloc], [1, CD]]))
            nc.sync.dma_start(out=mon_hi[HI_DUP:HI_DUP + 1, :],
                              in_=dap(x_dram, 0, [[F, 1], [1, F]]))

            # ---------------- m2F81 = xpre * xsuf
            xpre = big.tile([81, F], f16, name="xpre")
            xsuf = big.tile([81, F], f16, name="xsuf")
            HF = F // 2
            for h in range(2):
                hs = slice(h * HF, (h + 1) * HF)
                dma(xpre[:, hs],
                    dap(x_dram, h * HF, [[0, MD], [F, MD], [1, HF]]))
                dma(xsuf[:, hs],
                    dap(x_dram, h * HF, [[F, MD], [0, MD], [1, HF]]))
            for h in range(2):
                hs = slice(h * HF, (h + 1) * HF)
                nc.vector.tensor_mul(mon_hi[HI_M2:HI_M2 + 81, hs],
                                     xpre[:, hs], xsuf[:, hs])

            # ---------------- xrep (DRAM bcast) and m2rep (SBUF repeat)
            xrep_a = big.tile([128, F], f16, name="xrep_a")
            xrep_b = big.tile([N3 - 128, F], f16, name="xrep_b")
            for j in range(MD):
                ni, nk = j + 1, MD - j
                if BJ[j + 1] <= 128:
                    dma(xrep_a[BJ[j]:BJ[j + 1], :],
                        dap(x_dram, j * F, [[0, ni], [F, nk], [1, F]]))
                elif BJ[j] >= 128:
                    dma(xrep_b[BJ[j] - 128:BJ[j + 1] - 128, :],
                        dap(x_dram, j * F, [[0, ni], [F, nk], [1, F]]))
                else:
                    ia = (128 - BJ[j]) // nk
                    dma(xrep_a[BJ[j]:128, :],
                        dap(x_dram, j * F, [[0, ia], [F, nk], [1, F]]))
                    dma(xrep_b[0:BJ[j + 1] - 128, :],
                        dap(x_dram, j * F, [[0, ni - ia], [F, nk], [1, F]]))
            m2rep_a = big.tile([128, F], f16, name="m2rep_a")
            m2rep_b = big.tile([N3 - 128, F], f16, name="m2rep_b")
            for j in range(MD):
                ni, nk = j + 1, MD - j
                src0 = mon_hi.offset + (HI_M2 + 9 * j) * F
                if BJ[j + 1] <= 128:
                    dma(m2rep_a[BJ[j]:BJ[j + 1], :],
                        bass.AP(tensor=mon_hi.tensor, offset=src0,
                                ap=[[F, ni], [0, nk], [1, F]]))
                elif BJ[j] >= 128:
                    dma(m2rep_b[BJ[j] - 128:BJ[j + 1] - 128, :],
                        bass.AP(tensor=mon_hi.tensor, offset=src0,
                                ap=[[F, ni], [0, nk], [1, F]]))
                else:
                    ia = (128 - BJ[j]) // nk
                    dma(m2rep_a[BJ[j]:128, :],
                        bass.AP(tensor=mon_hi.tensor, offset=src0,
                                ap=[[F, ia], [0, nk], [1, F]]))
                    dma(m2rep_b[0:BJ[j + 1] - 128, :],
                        bass.AP(tensor=mon_hi.tensor, offset=src0 + ia * F,
                                ap=[[F, ni - ia], [0, nk], [1, F]]))

            # ---------------- m3 muls (DVE fast; Pool takes one slow half)
            m3scr = big.tile([N3 - 128, F], f16, name="m3scr")
            for h in range(2):
                hs = slice(h * HF, (h + 1) * HF)
                nc.vector.tensor_mul(mon_lo[:, hs], m2rep_a[:, hs],
                                     xrep_a[:, hs])
                eng = nc.vector if h == 0 else nc.gpsimd
                eng.tensor_mul(m3scr[:, hs], m2rep_b[:, hs], xrep_b[:, hs])
                dma(mon_hi[HI_M3B:HI_M3B + (N3 - 128), hs], m3scr[:, hs])

            # ---------------- main loop: D^T + fused w-apply/path-sum
            termF = big.tile([128, NJT * MD], f32, name="termF")
            termF16 = big.tile([128, NJT * MD], f16, name="termF16")
            with tc.tile_pool(name="ps_d", bufs=2, space="PSUM") as ps_d:
                for g in range(NGRP):
                    pt = ps_d.tile([128, NSLOT * SLOTW], f32, name="pt",
                                   tag="pt")
                    for k in range(NSLOT):
                        jt = g * NSLOT + k
                        js = slice(jt * 128, (jt + 1) * 128)
                        o_ = pt[:, k * SLOTW:k * SLOTW + MOUT]
                        nc.tensor.matmul(o_, lhsT=mon_lo[:, js], rhs=sb_uclo,
                                         start=True, stop=False)
                        nc.tensor.matmul(o_, lhsT=mon_hi[:, js], rhs=sb_uchi,
                                         start=False, stop=True)
                    dsb = work.tile([128, NSLOT * MOUT], f16, name="dsb",
                                    tag="dsb")
                    nc.scalar.activation(
                        dap(dsb, 0, [[NSLOT * MOUT, 128], [1, NSLOT * MOUT]]),
                        dap(pt, 0, [[NSLOT * SLOTW, 128], [SLOTW, NSLOT],
                                    [1, MOUT]]),
                        AF.Copy)
                    dwT = work.tile([128, NSLOT * MOUT], f16, name="dwT",
                                    tag="dwT")
                    nc.vector.tensor_mul(
                        dap(dwT, 0, [[NSLOT * MOUT, 128], [1, NSLOT * MOUT]]),
                        dsb[:],
                        dap(wgt, g * NSLOT * PPC,
                            [[NJT * PPC, 128], [PPC, NSLOT], [0, MD],
                             [1, PPC]]))
                    nc.vector.tensor_reduce(
                        dap(termF, g * NSLOT * MD,
                            [[NJT * MD, 128], [MD, NSLOT], [1, MD]]),
                        dap(dwT, 0, [[NSLOT * MOUT, 128], [MOUT, NSLOT],
                                     [PPC, MD], [1, PPC]]),
                        axis=AX.X, op=ALU.add)
            nc.scalar.activation(termF16[:], termF[:], AF.Copy)

            # ---------------- fc2 (block-diag over (u, c)) -> out
            outSB = big.tile([128, MD * CD], f32, name="outSB")
            with tc.tile_pool(name="ps_o", bufs=1, space="PSUM") as ps_o:
                out2 = ps_o.tile([128, MD * CD], f32, name="out2")
                # l-blocks in o-major cols: l0 -> cols 0:64, l1 -> 64:256,
                # l2 -> 256:512 (o=4..7) and 512:576 (o=8)
                mm = [(0, 0, 1, 0), (1, 1, 3, 64), (2, 4, 4, 256),
                      (2, 8, 1, 512)]
                for l, o0, no, c0 in mm:
                    rhs = dap(termF16, o0,
                              [[NJT * MD, 128], [1, no], [MD, NJT]])
                    nc.tensor.matmul(out2[:, c0:c0 + no * NJT],
                                     lhsT=sb_wc[:, l * 128:(l + 1) * 128],
                                     rhs=rhs, start=True, stop=True)
                nc.scalar.activation(outSB[:, 0:CD], out2[:, 0:CD],
                                     AF.Identity, bias=b2sb[:])
                nc.scalar.activation(outSB[:, CD:], out2[:, CD:], AF.Copy)
            nc.sync.dma_start(out=out_d, in_=outSB[:])

          if repeat > 1:
              with tc.For_i(0, repeat, 1):
                  _emit()
          else:
              _emit()

    return nc


def _get_program(nloc, repeat=1):
    key = (nloc, repeat)
    if key not in _PROGRAM:
        nc = _build_program(nloc, repeat)
        nc.compile()
        _PROGRAM[key] = nc
    return _PROGRAM[key]


# ---------------------------------------------------------------- host maps
def make_in_maps(irreps_x, atomic_numbers, w_fc1, b_fc1, U3, W3, U2, W2, U1,
                 W1, w_lin, w_fc2, b_fc2, nloc=NLOC, ncores=NCORES):
    irreps_x = np.asarray(irreps_x, np.float32)
    a_n = np.asarray(atomic_numbers).astype(np.int64)
    U3c, U2c = _sym_compress(np.asarray(U3, np.float64),
                             np.asarray(U2, np.float64))
    U_dev = _build_u_dev(U3c, U2c, np.asarray(U1, np.float32))
    w_comb = np.einsum('lde,lec->ldc', np.asarray(w_fc2, np.float32),
                       np.asarray(w_lin, np.float32))
    F = nloc * CD
    NJT = F // 128
    w9 = MD * nloc
    pkw = PK_BASE + w9

    # fc1 weights as rhs [65, 3*64]: rows c -> w1[l][d, c].T; row 64 = b1
    w1t = np.zeros((65, 3 * CD), np.float32)
    for l in range(3):
        w1t[0:CD, l * CD:(l + 1) * CD] = np.asarray(w_fc1, np.float32)[l].T
    w1t[64, 0:CD] = np.asarray(b_fc1, np.float32)
    # fc2 block-diag lhsT per l: [(u,c), (u,d)] = wc[d, c]
    wc = np.zeros((128, 3 * 128), np.float32)
    for l in range(3):
        for u in range(2):
            wc[u * CD:(u + 1) * CD, l * 128 + u * CD:l * 128 + (u + 1) * CD] \
                = w_comb[l].T
    b2 = np.concatenate([np.asarray(b_fc2, np.float32)] * 2)[:, None]

    w3g = np.asarray(W3, np.float32)[a_n]      # [N, 16, C]
    w2g = np.asarray(W2, np.float32)[a_n]
    w1g = np.asarray(W1, np.float32)[a_n]
    w22 = np.concatenate([w3g, w2g, w1g], axis=1)   # [N, 22, C]

    def put(buf, nm, arr):
        o = PK_OFF[nm]
        arr = np.asarray(arr, np.float32).astype(np.float16)
        buf[:arr.shape[0], o:o + arr.shape[1]] = arr

    in_maps = []
    for core in range(ncores):
        s = slice(core * nloc, (core + 1) * nloc)
        # xt [65, (m, n)]: xt[c, m*nloc + n] = x-after-?? raw input irreps
        xseg = irreps_x[s]                          # [nloc, 9, 64]
        xt = np.zeros((65, w9), np.float16)
        xt[0:CD] = xseg.transpose(2, 1, 0).reshape(CD, w9).astype(np.float16)
        xt[64, 0:nloc] = 1.0                        # ones row (bias, m=0)
        pk = np.zeros((128, pkw), np.float16)
        put(pk, "uclo", U_dev[0:128])
        put(pk, "uchi", U_dev[128:256])
        put(pk, "w1t", w1t)
        put(pk, "wc", wc)
        pk[0:65, PK_BASE:PK_BASE + w9] = xt
        # wgt [128, (jt, pp)]: partition p=(u,c); f = n*64+c, n = 2*jt+u
        wseg = w22[s]                               # [nloc, 22, C]
        wgt = np.zeros((128, NJT * PPC), np.float16)
        for u in range(2):
            # [jt, pp, c] -> partition u*64+c, col jt*22+pp
            wgt[u * CD:(u + 1) * CD] = (
                wseg[u::2].transpose(2, 0, 1)       # [C, NJT, 22]
                .reshape(CD, NJT * PPC).astype(np.float16))
        in_maps.append({"pk": pk, "b2": b2.astype(np.float32),
                        "wgt": wgt})
    return in_maps


def unpack_out(o, nloc=NLOC):
    # o [128, 576]: row u*64+d, col o_*64+jt  ->  [nloc, 9, 64]
    o = o.reshape(2, CD, MD, nloc // 2)             # [u, d, o, jt]
    return np.ascontiguousarray(o.transpose(3, 0, 2, 1)  # [jt, u, o, d]
                                .reshape(nloc, MD, CD)).astype(np.float32)


# ---------------------------------------------------------------- entry
def kernel(**inputs):
    from concourse import bass_utils
    in_maps = make_in_maps(**inputs)
    nc = _get_program(NLOC)
    res = bass_utils.run_bass_kernel_spmd(nc, in_maps,
                                          core_ids=list(range(NCORES)))
    outs = [unpack_out(res.results[c]["out"]) for c in range(NCORES)]
    return np.concatenate(outs, axis=0).astype(np.float32)
